# revision 12
# baseline (speedup 1.0000x reference)
"""Trainium2 Bass kernel for nn_MessagePassing (10-step 3x3 per-pixel-weighted stencil).

Algorithm (per core, one batch element):
  reference: nw = w / (sum_taps(w)+eps); 10x: x = sum_{di,dj} nw[di,dj] * shift(x, di, dj)

Device formulation (B-stationary, transpose-free): state lives as
[w=128 partitions, plane(h) x c] fp16.  For each output row r,
    out_r[wo, c] = sum_{di in 0..2} B[di, r]^T-free @ plane_{r+di-1}
realized as matmul(out, lhsT=B[di,r], rhs=plane) where
B[di,r][ws, wo] holds the RAW tap weight wt[3*di+dj, r, wo] at
ws = wo+dj-1 (host-side pure scatter/layout, fp16).  The three di
matmuls accumulate in PSUM; the output lands directly in state
orientation [wo, c] so no per-step transposes are needed.
Normalization (1/(sum9+eps), computed on device in fp32) is folded
into the PSUM->SBUF evacuation as a broadcast multiply: each psum
bank (8 rows) is multiplied by rec[w, r] (free-broadcast over c)
while casting f32 -> f16 into the next state buffer.

Layouts (per core):
  state: [w=128, (H+2) planes x 64 c] fp16 x2 ping-pong; plane 0 and
         plane 129 are zero pads (the 3x3 boundary).
  B:     16 chunk tiles [ws=128, (8 rows) x (di 3) x (wo 128)] fp16.
  rec:   [w=128, h=128] f32 = 1/(sum of 9 taps + eps), from wt9T.
  psum:  [128, 512] f32 = one bank = 8 output rows x 64 c.
"""

import numpy as np

C, H, W = 64, 128, 128
N_CORES = 8
STEPS = 10
EPS = 1e-5
RPB = 8            # output rows per psum bank / evac group
NG = H // RPB      # 16 groups per step
HCH = 8            # h rows per B chunk tile
NBC = H // HCH     # 16 B chunks
PL = H + 2         # state planes incl. zero pads


def build_nc():
    import concourse.mybir as mybir
    from concourse import bacc
    from concourse.tile import TileContext

    f32 = mybir.dt.float32
    f16 = mybir.dt.float16

    nc = bacc.Bacc(trn_type="TRN2", target_bir_lowering=False, debug=False)
    # chunk-major layouts: each DMA chunk is a contiguous block of DRAM
    xT = nc.dram_tensor("xT", [NG * W, RPB * C], f16, kind="ExternalInput").ap()
    braw = nc.dram_tensor("braw", [NBC * W, HCH * 3 * W], f16,
                          kind="ExternalInput").ap()
    wt9T = nc.dram_tensor("wt9T", [W, H * 9], f32, kind="ExternalInput").ap()
    yT = nc.dram_tensor("yT", [NG * W, RPB * C], f16, kind="ExternalOutput").ap()

    with TileContext(nc) as tc:
        with (
            tc.tile_pool(name="per", bufs=1) as per,
            tc.tile_pool(name="ps", bufs=8, space="PSUM") as ps,
        ):
            # ---- persistent SBUF ----
            Bt = [per.tile([W, HCH * 3 * W], f16, tag=f"B{k}", name=f"B{k}")
                  for k in range(NBC)]
            st = [per.tile([W, PL * C], f16, tag=f"st{s}", name=f"st{s}")
                  for s in range(2)]
            wt_sb = per.tile([W, H * 9], f32, tag="wt")
            sum9 = per.tile([W, H], f32, tag="sum9")
            rec = per.tile([W, H], f32, tag="rec")

            # ---- loads ----
            # even B chunks on the scalar HWDGE queue; wt9T, x, then odd B
            # chunks on the sync HWDGE queue (two queues stream concurrently)
            for k in range(0, NBC, 2):
                nc.scalar.dma_start(
                    out=Bt[k][:], in_=braw[k * W:(k + 1) * W, :]
                )
            nc.sync.dma_start(out=wt_sb[:], in_=wt9T)
            # initial state, in row groups so step 1 can start as soon as
            # the first groups land
            for g in range(NG):
                nc.sync.dma_start(
                    out=st[0][:, (g * RPB + 1) * C:(g * RPB + RPB + 1) * C],
                    in_=xT[g * W:(g + 1) * W, :],
                )
            for k in range(1, NBC, 2):
                nc.sync.dma_start(
                    out=Bt[k][:], in_=braw[k * W:(k + 1) * W, :]
                )
            # zero pads (both ping-pong buffers, never written again)
            for s in range(2):
                nc.vector.memset(st[s][:, 0:C], 0.0)
                nc.vector.memset(st[s][:, (H + 1) * C:PL * C], 0.0)

            # ---- rec = 1/(sum9 + eps), in [w, h] orientation ----
            nc.vector.tensor_reduce(
                out=sum9[:].unsqueeze(2),
                in_=wt_sb[:].rearrange("p (h t) -> p h t", t=9),
                axis=mybir.AxisListType.X,
                op=mybir.AluOpType.add,
            )
            nc.vector.tensor_scalar_add(out=sum9[:], in0=sum9[:],
                                        scalar1=float(EPS))
            nc.vector.reciprocal(out=rec[:], in_=sum9[:])

            # ---- helper ----
            def bmat(r, di):  # stationary [ws=128, wo=128] for (row r, di)
                k, rr = divmod(r, HCH)
                off = (rr * 3 + di) * W
                return Bt[k][:, off:off + W]

            # ---- the 10 steps, emitted in wavefront order ----
            # Engines execute their instruction streams in program order, so
            # emit (step, group) pairs in a dependency-feasible wavefront:
            # step s group g only needs step s-1 groups <= g+1 and B chunk g.
            # This lets deeper steps run on already-resident B chunks while
            # step 1 still waits for its (DMA-gated) later chunks.
            def do_group(s, g):
                src = st[s % 2]
                dst = st[(s + 1) % 2]
                pst = ps.tile([W, RPB * C], f32, tag="ps", name="ps")
                for rr in range(RPB):
                    r = g * RPB + rr
                    for di in range(3):
                        # source plane index r+di (zero pads at the ends
                        # make the boundary rows correct)
                        nc.tensor.matmul(
                            out=pst[:, rr * C:(rr + 1) * C],
                            lhsT=bmat(r, di),
                            rhs=src[:, (r + di) * C:(r + di + 1) * C],
                            start=(di == 0),
                            stop=(di == 2),
                            # 8 independent row-groups share this bank;
                            # per-element pending-zero semantics make
                            # this correct but the sim's group checker
                            # conflates col ranges within a bank.
                            skip_group_check=True,
                        )
                # fused evacuation: cast f32->f16 AND normalize by
                # rec[w, r] (broadcast over c)
                in0 = pst[:].rearrange("p (r c) -> p r c", r=RPB)
                in1 = (rec[:, g * RPB:(g + 1) * RPB]
                       .unsqueeze(2).broadcast_to([W, RPB, C]))
                outap = dst[:, (g * RPB + 1) * C:(g * RPB + RPB + 1) * C
                            ].rearrange("p (r c) -> p r c", r=RPB)
                nc.vector.tensor_mul(out=outap, in0=in0, in1=in1)
                if s == STEPS - 1:
                    # stream the finished group straight out (fp16, HWDGE)
                    nc.sync.dma_start(
                        out=yT[g * W:(g + 1) * W, :],
                        in_=dst[:, (g * RPB + 1) * C:(g * RPB + RPB + 1) * C],
                    )

            for w in range(NG + 2 * (STEPS - 1)):
                for s in range(STEPS):
                    g = w - 2 * s
                    if 0 <= g < NG:
                        do_group(s, g)

    if not nc.is_finalized():
        nc.finalize()
    return nc


def host_prep(inp_i, wt_i):
    """Per-core host-side layout transforms (+ the fp16 quantization the
    device pipeline uses; the f16->f32 widening on output is exact)."""
    # xT chunk-major: [g, w, r, c] = x[c, 8g+r, w]
    xT = (inp_i.transpose(2, 1, 0)          # [w, h, c]
          .reshape(W, NG, RPB * C)
          .transpose(1, 0, 2)               # [g, w, r*c]
          .reshape(NG * W, RPB * C))
    # braw[ws, h, di, wo] = wt_i[3*di+dj, h, wo] with ws = wo+dj-1
    braw = np.zeros((W, H, 3, W), dtype=np.float16)
    wo = np.arange(W)
    for di in range(3):
        for dj in range(3):
            ws = wo + dj - 1
            m = (ws >= 0) & (ws < W)
            braw[ws[m], :, di, wo[m]] = wt_i[3 * di + dj][:, wo[m]].T.astype(
                np.float16
            )
    # chunk-major: [k, ws, (rows-of-chunk) x 3 x wo]
    braw = (braw.reshape(W, NBC, HCH * 3 * W)
            .transpose(1, 0, 2)
            .reshape(NBC * W, HCH * 3 * W))
    braw = np.ascontiguousarray(braw)
    # wt9T[w, h, t] = wt_i[t, h, w]
    wt9T = np.ascontiguousarray(wt_i.transpose(2, 1, 0)).reshape(W, H * 9)
    return {
        "xT": np.ascontiguousarray(xT).astype(np.float16),
        "braw": braw,
        "wt9T": wt9T.astype(np.float32),
    }


def unpack(yT):
    # yT[g, w, r, c] -> [c, h=8g+r, w]
    return (yT.reshape(NG, W, RPB, C)
            .transpose(3, 0, 2, 1)
            .reshape(C, H, W)
            .astype(np.float32))


LAST_RESULTS = None  # BassKernelResults of the most recent kernel() call


def kernel(**inputs):
    import os
    from concourse.bass_utils import run_bass_kernel_spmd

    global LAST_RESULTS
    inp = np.asarray(inputs["input"], dtype=np.float32)
    wt = np.asarray(inputs["weight"], dtype=np.float32)
    n = inp.shape[0]
    in_maps = [host_prep(inp[i], wt[i]) for i in range(n)]
    nc = build_nc()
    trace = bool(int(os.environ.get("MP_TRACE", "0")))
    res = run_bass_kernel_spmd(
        nc, in_maps, core_ids=list(range(n)), trace=trace
    )
    LAST_RESULTS = res
    out = np.stack([unpack(r["yT"]) for r in res.results])
    return out.astype(np.float32)


if __name__ == "__main__":
    nc = build_nc()
    print("built ok")


# revision 17
# speedup vs baseline: 1.0740x; 1.0740x over previous
"""Trainium2 Bass kernel for nn_MessagePassing (10-step 3x3 per-pixel-weighted stencil).

Algorithm (per core, one batch element):
  reference: nw = w / (sum_taps(w)+eps); 10x: x = sum_{di,dj} nw[di,dj] * shift(x, di, dj)

Device formulation (B-stationary, transpose-free): state lives as
[w=128 partitions, plane(h) x c] fp16.  For each output row r,
    out_r[wo, c] = sum_{di in 0..2} B[di, r]^T-free @ plane_{r+di-1}
realized as matmul(out, lhsT=B[di,r], rhs=plane) where
B[di,r][ws, wo] holds the RAW tap weight wt[3*di+dj, r, wo] at
ws = wo+dj-1 (host-side pure scatter/layout, fp16).  The three di
matmuls accumulate in PSUM; the output lands directly in state
orientation [wo, c] so no per-step transposes are needed.
Normalization (1/(sum9+eps), computed on device in fp32) is folded
into the PSUM->SBUF evacuation as a broadcast multiply: each psum
bank (8 rows) is multiplied by rec[w, r] (free-broadcast over c)
while casting f32 -> f16 into the next state buffer.

Layouts (per core):
  state: [w=128, (H+2) planes x 64 c] fp16 x2 ping-pong; plane 0 and
         plane 129 are zero pads (the 3x3 boundary).
  B:     16 chunk tiles [ws=128, (8 rows) x (di 3) x (wo 128)] fp16.
  rec:   [w=128, h=128] f32 = 1/(sum of 9 taps + eps), from wt9T.
  psum:  [128, 512] f32 = one bank = 8 output rows x 64 c.
"""

import numpy as np

C, H, W = 64, 128, 128
N_CORES = 8
STEPS = 10
EPS = 1e-5
RPB = 8            # output rows per psum bank / evac group
NG = H // RPB      # 16 groups per step
HCH = 8            # h rows per B chunk tile
NBC = H // HCH     # 16 B chunks
PL = H + 2         # state planes incl. zero pads
XCH = 4            # x groups per input DMA transfer
NXQ = NG // XCH    # 4 input transfers


def build_nc():
    import concourse.mybir as mybir
    from concourse import bacc
    from concourse.tile import TileContext

    f32 = mybir.dt.float32
    f16 = mybir.dt.float16

    nc = bacc.Bacc(trn_type="TRN2", target_bir_lowering=False, debug=False)
    # chunk-major layouts: each DMA chunk is a contiguous block of DRAM
    xT = nc.dram_tensor("xT", [NXQ * W, XCH * RPB * C], f16,
                        kind="ExternalInput").ap()
    braw = nc.dram_tensor("braw", [NBC * W, HCH * 3 * W], f16,
                          kind="ExternalInput").ap()
    wt9T = nc.dram_tensor("wt9T", [W, H * 9], f32, kind="ExternalInput").ap()
    yT = nc.dram_tensor("yT", [NG * W, RPB * C], f16, kind="ExternalOutput").ap()

    with TileContext(nc) as tc:
        with (
            tc.tile_pool(name="per", bufs=1) as per,
            tc.tile_pool(name="ps", bufs=8, space="PSUM") as ps,
        ):
            # ---- persistent SBUF ----
            Bt = [per.tile([W, HCH * 3 * W], f16, tag=f"B{k}", name=f"B{k}")
                  for k in range(NBC)]
            st = [per.tile([W, PL * C], f16, tag=f"st{s}", name=f"st{s}")
                  for s in range(2)]
            wt_sb = per.tile([W, H * 9], f32, tag="wt")
            sum9 = per.tile([W, H], f32, tag="sum9")
            rec = per.tile([W, H], f32, tag="rec")

            # ---- loads ----
            # B chunks stream on the scalar HWDGE queue (contiguous 768KB
            # each); wt9T + x go on the sync queue in a few large transfers
            # so neither queue is clogged with small descriptors.
            for k in range(NBC):
                nc.scalar.dma_start(
                    out=Bt[k][:], in_=braw[k * W:(k + 1) * W, :]
                )
            nc.sync.dma_start(out=wt_sb[:], in_=wt9T)
            for q in range(NXQ):
                nc.sync.dma_start(
                    out=st[0][:, (q * XCH * RPB + 1) * C:
                              ((q + 1) * XCH * RPB + 1) * C],
                    in_=xT[q * W:(q + 1) * W, :],
                )
            # zero pads (both ping-pong buffers, never written again)
            for s in range(2):
                nc.vector.memset(st[s][:, 0:C], 0.0)
                nc.vector.memset(st[s][:, (H + 1) * C:PL * C], 0.0)

            # ---- rec = 1/(sum9 + eps), in [w, h] orientation ----
            nc.vector.tensor_reduce(
                out=sum9[:].unsqueeze(2),
                in_=wt_sb[:].rearrange("p (h t) -> p h t", t=9),
                axis=mybir.AxisListType.X,
                op=mybir.AluOpType.add,
            )
            nc.vector.tensor_scalar_add(out=sum9[:], in0=sum9[:],
                                        scalar1=float(EPS))
            nc.vector.reciprocal(out=rec[:], in_=sum9[:])

            # ---- helper ----
            def bmat(r, di):  # stationary [ws=128, wo=128] for (row r, di)
                k, rr = divmod(r, HCH)
                off = (rr * 3 + di) * W
                return Bt[k][:, off:off + W]

            # ---- the 10 steps, emitted in wavefront order ----
            # Engines execute their instruction streams in program order, so
            # emit (step, group) pairs in a dependency-feasible wavefront:
            # step s group g only needs step s-1 groups <= g+1 and B chunk g.
            # This lets deeper steps run on already-resident B chunks while
            # step 1 still waits for its (DMA-gated) later chunks.
            def do_group(s, g):
                src = st[s % 2]
                dst = st[(s + 1) % 2]
                pst = ps.tile([W, RPB * C], f32, tag="ps", name="ps")
                for rr in range(RPB):
                    r = g * RPB + rr
                    for di in range(3):
                        # source plane index r+di (zero pads at the ends
                        # make the boundary rows correct)
                        nc.tensor.matmul(
                            out=pst[:, rr * C:(rr + 1) * C],
                            lhsT=bmat(r, di),
                            rhs=src[:, (r + di) * C:(r + di + 1) * C],
                            start=(di == 0),
                            stop=(di == 2),
                            # 8 independent row-groups share this bank;
                            # per-element pending-zero semantics make
                            # this correct but the sim's group checker
                            # conflates col ranges within a bank.
                            skip_group_check=True,
                        )
                # fused evacuation: cast f32->f16 AND normalize by
                # rec[w, r] (broadcast over c)
                in0 = pst[:].rearrange("p (r c) -> p r c", r=RPB)
                in1 = (rec[:, g * RPB:(g + 1) * RPB]
                       .unsqueeze(2).broadcast_to([W, RPB, C]))
                outap = dst[:, (g * RPB + 1) * C:(g * RPB + RPB + 1) * C
                            ].rearrange("p (r c) -> p r c", r=RPB)
                nc.vector.tensor_mul(out=outap, in0=in0, in1=in1)
                if s == STEPS - 1:
                    # stream the finished group straight out (fp16, HWDGE)
                    nc.sync.dma_start(
                        out=yT[g * W:(g + 1) * W, :],
                        in_=dst[:, (g * RPB + 1) * C:(g * RPB + RPB + 1) * C],
                    )

            for w in range(NG + 2 * (STEPS - 1)):
                for s in range(STEPS):
                    g = w - 2 * s
                    if 0 <= g < NG:
                        do_group(s, g)

    if not nc.is_finalized():
        nc.finalize()
    return nc


def host_prep(inp_i, wt_i):
    """Per-core host-side layout transforms (+ the fp16 quantization the
    device pipeline uses; the f16->f32 widening on output is exact)."""
    # xT chunk-major: [q, w, (32 rows) x c] = x[c, 32q+rr, w]
    xT = (inp_i.transpose(2, 1, 0)          # [w, h, c]
          .reshape(W, NXQ, XCH * RPB * C)
          .transpose(1, 0, 2)
          .reshape(NXQ * W, XCH * RPB * C))
    # braw[ws, h, di, wo] = wt_i[3*di+dj, h, wo] with ws = wo+dj-1
    braw = np.zeros((W, H, 3, W), dtype=np.float16)
    wo = np.arange(W)
    for di in range(3):
        for dj in range(3):
            ws = wo + dj - 1
            m = (ws >= 0) & (ws < W)
            braw[ws[m], :, di, wo[m]] = wt_i[3 * di + dj][:, wo[m]].T.astype(
                np.float16
            )
    # chunk-major: [k, ws, (rows-of-chunk) x 3 x wo]
    braw = (braw.reshape(W, NBC, HCH * 3 * W)
            .transpose(1, 0, 2)
            .reshape(NBC * W, HCH * 3 * W))
    braw = np.ascontiguousarray(braw)
    # wt9T[w, h, t] = wt_i[t, h, w]
    wt9T = np.ascontiguousarray(wt_i.transpose(2, 1, 0)).reshape(W, H * 9)
    return {
        "xT": np.ascontiguousarray(xT).astype(np.float16),
        "braw": braw,
        "wt9T": wt9T.astype(np.float32),
    }


def unpack(yT):
    # yT[g, w, r, c] -> [c, h=8g+r, w]
    return (yT.reshape(NG, W, RPB, C)
            .transpose(3, 0, 2, 1)
            .reshape(C, H, W)
            .astype(np.float32))


LAST_RESULTS = None  # BassKernelResults of the most recent kernel() call


def kernel(**inputs):
    import os
    from concourse.bass_utils import run_bass_kernel_spmd

    global LAST_RESULTS
    inp = np.asarray(inputs["input"], dtype=np.float32)
    wt = np.asarray(inputs["weight"], dtype=np.float32)
    n = inp.shape[0]
    in_maps = [host_prep(inp[i], wt[i]) for i in range(n)]
    nc = build_nc()
    trace = bool(int(os.environ.get("MP_TRACE", "0")))
    res = run_bass_kernel_spmd(
        nc, in_maps, core_ids=list(range(n)), trace=trace
    )
    LAST_RESULTS = res
    out = np.stack([unpack(r["yT"]) for r in res.results])
    return out.astype(np.float32)


if __name__ == "__main__":
    nc = build_nc()
    print("built ok")
